# revision 1
# baseline (speedup 1.0000x reference)
"""Bass/Trainium2 kernel for ExtendedTripletLoss (data-parallel over batch).

Math: for each pair (f1,m1),(f2,m2) and shift off in [-4,4]:
  num(off) = sum mask*(f1-f2r)^2 = t1 + t2 - 2*t3
    t1 = corr(A, m2)(off),   A  = sum_c (m1*f1)^2        [32,512]
    t2 = corr(m1, B2)(off),  B2 = sum_c (m2*f2)^2        [32,512]
    t3 = corr(U, V)(off),    U = m1*f1, V = m2*f2        [512,512]
  den(off) = C * corr(m1, m2)(off) + 1e-3
All correlations at 9 lags are computed on TensorE as Gram-block matmuls:
contraction over rows (c,h), w blocked 4x128; rhs uses a +-4 padded copy so
each block's 136-wide window holds all 9 shifted columns. All 4 w-blocks and
all terms accumulate into ONE PSUM tile [128,136]; lag sums are the 9
diagonals col = i + 4 - off, extracted on the host from the DMA'd blocks.
"""

import os
import sys
from contextlib import ExitStack

import numpy as np

for _p in ("/opt/trn_rl_repo", "/root/.axon_site/_ro/trn_rl_repo"):
    if os.path.isdir(_p) and _p not in sys.path:
        sys.path.insert(0, _p)
        break

import ml_dtypes

import concourse.bass as bass
import concourse.mybir as mybir
import concourse.tile as tile
# This environment's walrus_driver allows only ONE sync-wait per instruction,
# while Tile freely aggregates several. Post-pass: move excess waits onto
# freshly inserted same-engine NOPs directly before the instruction.
_MAXW = 1


def _split_waits_pass(nc):
    n = 0
    for fn in nc.m.functions:
        for blk in fn.blocks:
            out = []
            changed = False
            for inst in blk.instructions:
                si = inst.sync_info
                waits = list(si.on_wait) if si is not None else []
                if len(waits) > _MAXW:
                    for i in range(0, len(waits) - _MAXW, _MAXW):
                        nop = mybir.InstNoOp(name=f"{inst.name}-wsplit{i}")
                        nop.engine = inst.engine
                        nop.sync_info = mybir.SyncInfo(
                            on_update=[], on_wait=waits[i : i + _MAXW]
                        )
                        out.append(nop)
                        n += 1
                    si.on_wait = waits[len(waits) - _MAXW :]
                    changed = True
                out.append(inst)
            if changed:
                blk.instructions = out
    return n


# concourse pins --enable-ldw-opt=false; enabling lets walrus elide/overlap
# redundant weight loads, which are ~30% of this kernel's PE time.
def _patch_ldw_opt():
    from concourse import bass_utils as _bu

    if getattr(_bu, "_ldw_opt_patched", False):
        return
    _orig = _bu.run_command

    def _run_command_ldwopt(cmd, *a, **kw):
        if isinstance(cmd, list):
            cmd = [
                "--enable-ldw-opt=true" if c == "--enable-ldw-opt=false" else c
                for c in cmd
            ]
        return _orig(cmd, *a, **kw)

    _bu.run_command = _run_command_ldwopt
    _bu._ldw_opt_patched = True


if os.environ.get("BASS_LDW_OPT", "0") == "1":
    _patch_ldw_opt()

BF16 = mybir.dt.bfloat16
F32 = mybir.dt.float32

B, C, H, W = 64, 16, 32, 512
NCORES = 8
S = B // NCORES          # samples per core
R = C * H                # 512 rows in (c,h) contraction dim
NB = R // 128            # 4 partition chunks
JB = W // 128            # 4 w-blocks
NW = 136                 # window width = 128 + 2*4
MARGIN = 0.15
SHIFT = 4

_nc_cache = None


def build_nc(for_hw=True):
    nc = bass.Bass()
    x_a = nc.declare_dram_parameter("x_a", [S, R, W], BF16, isOutput=False)
    x_p = nc.declare_dram_parameter("x_p", [S, R, W], BF16, isOutput=False)
    x_n = nc.declare_dram_parameter("x_n", [S, R, W], BF16, isOutput=False)
    # masks_ext: circularly padded along W: [:, 0:4]=m[:, 508:512],
    # [:, 4:516]=m, [:, 516:520]=m[:, 0:4]; rows = [ma; mp; mn]
    masks = nc.declare_dram_parameter("masks", [S, 3 * H, W + 8], BF16, isOutput=False)
    # mask replicas: [s, p, t, w] = mask_t[p % 32, w], with t=1,2 pre-scaled by -2
    masks_rep = nc.declare_dram_parameter("masks_rep", [S, 128, 3, W], BF16, isOutput=False)
    ind = nc.declare_dram_parameter("ind", [128, H], BF16, isOutput=False)
    # 0.25-scaled indicator: folds the (-2)^2 of the pre-scaled masks out of
    # the Bp/Bn channel-reductions (exact: power of two)
    ind4 = nc.declare_dram_parameter("ind4", [128, H], BF16, isOutput=False)
    # raw[s, i, g, c]: g = (num-ap, num-an); den is host-computed from masks
    raw = nc.declare_dram_parameter("raw", [S, 128, 2, NW], F32, isOutput=True)

    with tile.TileContext(nc) as tc, ExitStack() as ctx:
        const = ctx.enter_context(tc.tile_pool(name="const", bufs=1))
        io = ctx.enter_context(tc.tile_pool(name="io", bufs=3))
        mk = ctx.enter_context(tc.tile_pool(name="mk", bufs=3))
        um = ctx.enter_context(tc.tile_pool(name="um", bufs=3))
        sq = ctx.enter_context(tc.tile_pool(name="sq", bufs=3))
        k4p = ctx.enter_context(tc.tile_pool(name="k4p", bufs=3))
        outsb = ctx.enter_context(tc.tile_pool(name="outsb", bufs=4))
        indps = ctx.enter_context(tc.tile_pool(name="indps", bufs=3, space="PSUM"))
        gram = ctx.enter_context(tc.tile_pool(name="gram", bufs=2, space="PSUM"))

        ind_sb = const.tile([128, H], BF16)
        nc.sync.dma_start(out=ind_sb, in_=ind[:])
        ind4_sb = const.tile([128, H], BF16)
        nc.sync.dma_start(out=ind4_sb, in_=ind4[:])

        # PE prewarm: ~4us of junk matmuls so the HAM un-throttles during
        # the pipeline-fill phase instead of during the first real samples.
        warm_ps = ctx.enter_context(
            tc.tile_pool(name="warm", bufs=1, space="PSUM")
        ).tile([H, H], F32)
        for _ in range(60):
            nc.tensor.matmul(warm_ps, ind_sb, ind_sb[:, 0:H], start=True, stop=True)

        mult = mybir.AluOpType.mult

        for s in range(S):
            # ---- loads ----
            abuf = io.tile([128, NB, W], BF16, tag="abuf")
            pn = io.tile([128, NB, 2, W], BF16, tag="pn")
            nc.sync.dma_start(out=abuf, in_=x_a[s].rearrange("(j p) w -> p j w", p=128))
            nc.sync.dma_start(
                out=pn[:, :, 0, :], in_=x_p[s].rearrange("(j p) w -> p j w", p=128)
            )
            nc.sync.dma_start(
                out=pn[:, :, 1, :], in_=x_n[s].rearrange("(j p) w -> p j w", p=128)
            )

            # ---- mask replicas to 128 partitions: one broadcast DMA each ----
            mrep = mk.tile([128, 3, W], BF16, tag="mrep")
            nc.gpsimd.dma_start(out=mrep, in_=masks_rep[s])

            def rep_b(t):
                # [128, NB, W] view of mrep[:, t, :] broadcast over the NB axis
                return mrep[:, t, :].unsqueeze(1).broadcast_to((128, NB, W))

            # ---- masking (DVE): U = ma*a ; vw = [-2*mp*p | -2*mn*n] ----
            ubuf = um.tile([128, NB, W], BF16, tag="ubuf")
            vw = um.tile([128, NB, 2, W + 8], BF16, tag="vw")
            nc.vector.tensor_tensor(out=ubuf, in0=abuf, in1=rep_b(0), op=mult)
            nc.vector.tensor_tensor(
                out=vw[:, :, :, 4 : W + 4],
                in0=pn,
                in1=mrep[:, 1:3, :].unsqueeze(1).broadcast_to((128, NB, 2, W)),
                op=mult,
            )
            # circular wrap columns (both pairs at once)
            nc.vector.tensor_copy(out=vw[:, :, :, 0:4], in_=vw[:, :, :, W : W + 4])
            nc.vector.tensor_copy(out=vw[:, :, :, W + 4 : W + 8], in_=vw[:, :, :, 4:8])

            # ---- squares: u2 on ACT (Square is 1x there; small one goes to
            # ACT, big one to DVE where TT-mul runs 2x) ----
            u2 = sq.tile([128, NB, W], BF16, tag="u2")
            vw2 = sq.tile([128, NB, 2, W], BF16, tag="vw2")
            Sq = mybir.ActivationFunctionType.Square
            nc.scalar.activation(out=u2, in_=ubuf, func=Sq)
            nc.vector.tensor_tensor(
                out=vw2,
                in0=vw[:, :, :, 4 : W + 4],
                in1=vw[:, :, :, 4 : W + 4],
                op=mult,
            )

            # ---- c-reduction via indicator matmul: A/Bp/Bn [32, 512] ----
            a_ps = indps.tile([H, W], F32, tag="ind3")
            b_ps = indps.tile([H, W], F32, tag="ind3")
            c_ps = indps.tile([H, W], F32, tag="ind3")
            for j in range(NB):
                nc.tensor.matmul(a_ps, ind_sb, u2[:, j, :], start=(j == 0), stop=(j == NB - 1))
            for j in range(NB):
                nc.tensor.matmul(b_ps, ind4_sb, vw2[:, j, 0, :], start=(j == 0), stop=(j == NB - 1))
            for j in range(NB):
                nc.tensor.matmul(c_ps, ind4_sb, vw2[:, j, 1, :], start=(j == 0), stop=(j == NB - 1))

            # ---- assemble k4 lhsT [A; ma] and rhs [m2_ext; B2_ext] x pairs ----
            Cp = mybir.ActivationFunctionType.Copy
            k4lhs = k4p.tile([2 * H, W], BF16, tag="k4lhs")
            r44 = k4p.tile([2 * H, 2, W + 8], BF16, tag="r44")
            nc.scalar.activation(out=k4lhs[0:H, :], in_=a_ps, func=Cp)
            nc.gpsimd.dma_start(
                out=k4lhs[H : 2 * H, :], in_=masks[s, 0:H, 4 : W + 4]
            )
            nc.gpsimd.dma_start(
                out=r44[0:H, :, :],
                in_=masks[s, H : 3 * H, :].rearrange("(t p) w -> p t w", p=H),
            )
            nc.scalar.activation(out=r44[H : 2 * H, 0, 4 : W + 4], in_=b_ps, func=Cp)
            nc.scalar.activation(out=r44[H : 2 * H, 1, 4 : W + 4], in_=c_ps, func=Cp)
            nc.vector.tensor_copy(
                out=r44[H : 2 * H, :, 0:4], in_=r44[H : 2 * H, :, W : W + 4]
            )
            nc.vector.tensor_copy(
                out=r44[H : 2 * H, :, W + 4 : W + 8], in_=r44[H : 2 * H, :, 4:8]
            )

            # ---- Gram matmuls (both pairs per matmul via 3D rhs) ----
            num_ps = gram.tile([128, 2, NW], F32, tag="num")
            for j1 in range(JB):
                mb = slice(j1 * 128, (j1 + 1) * 128)
                wn = slice(j1 * 128, j1 * 128 + NW)
                for kc in range(NB):
                    nc.tensor.matmul(
                        num_ps, ubuf[:, kc, mb], vw[:, kc, :, wn],
                        start=(j1 == 0 and kc == 0), stop=False,
                    )
                nc.tensor.matmul(
                    num_ps, k4lhs[:, mb], r44[:, :, wn],
                    start=False, stop=(j1 == JB - 1),
                )
            psb = outsb.tile([128, 2, NW], F32, tag="psb")
            nc.scalar.activation(out=psb, in_=num_ps, func=Cp)
            nc.gpsimd.dma_start(out=raw[s], in_=psb)
    if for_hw:
        _split_waits_pass(nc)
    return nc


def _host_prep(a, p, n, ma, mp, mn):
    bf = ml_dtypes.bfloat16
    A = np.ascontiguousarray(a.reshape(B, R, W)).astype(bf)
    P = np.ascontiguousarray(p.reshape(B, R, W)).astype(bf)
    N = np.ascontiguousarray(n.reshape(B, R, W)).astype(bf)
    M0 = np.concatenate(
        [ma.reshape(B, H, W), mp.reshape(B, H, W), mn.reshape(B, H, W)], axis=1
    ).astype(bf)
    M = np.concatenate([M0[:, :, W - 4 :], M0, M0[:, :, :4]], axis=2)
    # replicas: [b, p, t, w] = mask_t[p % 32, w]; mp/mn rows pre-scaled by -2
    Mr = np.stack(
        [
            np.tile(ma.reshape(B, H, W), (1, 4, 1)),
            np.tile(mp.reshape(B, H, W).astype(np.float32) * -2.0, (1, 4, 1)),
            np.tile(mn.reshape(B, H, W).astype(np.float32) * -2.0, (1, 4, 1)),
        ],
        axis=2,
    ).astype(bf)
    ind = np.zeros((128, H), dtype=bf)
    ind[np.arange(128), np.arange(128) % H] = 1
    ind4 = np.zeros((128, H), dtype=bf)
    ind4[np.arange(128), np.arange(128) % H] = 0.25
    in_maps = []
    for c in range(NCORES):
        sl = slice(c * S, (c + 1) * S)
        in_maps.append(
            {
                "x_a": A[sl],
                "x_p": P[sl],
                "x_n": N[sl],
                "masks": M[sl],
                "masks_rep": Mr[sl],
                "ind": ind,
                "ind4": ind4,
            }
        )
    return in_maps


def _host_den(ma, mp, mn):
    # den counts[b, pair, off] = sum(m1 & roll(m2, off, -1)) over (1,2,3)
    nb = ma.shape[0]
    m1 = ma.reshape(nb, H, W).astype(bool)
    cnts = np.empty((nb, 2, 2 * SHIFT + 1), np.float64)
    for pair, m2 in enumerate((mp, mn)):
        m2 = m2.reshape(nb, H, W).astype(bool)
        for i, off in enumerate(range(-SHIFT, SHIFT + 1)):
            cnts[:, pair, i] = (m1 & np.roll(m2, off, axis=-1)).sum(axis=(1, 2))
    return cnts


def _host_finish(raw_all, cnts):
    # raw_all: [B, 128, 2, NW] float32; g = (num-ap, num-an)
    raw64 = raw_all.astype(np.float64)
    nums = raw64.transpose(0, 2, 1, 3)             # [B, 2, 128, NW]
    idx = np.arange(128)
    dists = []
    for i, off in enumerate(range(-SHIFT, SHIFT + 1)):
        cols = idx + 4 - off
        num = nums[:, :, idx, cols].sum(axis=-1)   # [B, 2]
        dists.append(num / (C * cnts[:, :, i] + 0.001))
    d = np.min(np.stack(dists, axis=0), axis=0)    # [B, 2]
    loss = np.maximum(d[:, 0] - d[:, 1] + MARGIN, 0.0)
    return np.array(loss.mean(), dtype=np.float32)


def kernel(a, p, n, ma, mp, mn):
    global _nc_cache
    from concourse import bass_utils

    if _nc_cache is None:
        _nc_cache = build_nc()
    nc = _nc_cache
    in_maps = _host_prep(a, p, n, ma, mp, mn)
    res = bass_utils.run_bass_kernel_spmd(nc, in_maps, core_ids=list(range(NCORES)))
    raw_all = np.concatenate([res.results[i]["raw"] for i in range(NCORES)], axis=0)
    return _host_finish(raw_all, _host_den(ma, mp, mn))



# revision 8
# speedup vs baseline: 1.0589x; 1.0589x over previous
"""Bass/Trainium2 kernel for ExtendedTripletLoss (data-parallel over batch).

FP8 redesign. Math per sample and shift off in [-4,4], pair g in {ap, an}:
  num(off) = t1 + t2 - 2*t3
    t3 = sum_{r,w} U[r,w] V[r,w-off],  U = m1*f1, V = m2*f2   (fp8)
    t1 = sum_{h,w} m1[h,w] m2[h,w-off] P1[h,w],   P1 = sum_c f1^2
    t2 = sum_{h,w} m1[h,w] m2[h,w-off] P2[h,w-off], P2 = sum_c f2^2
  den(off) = C * |m1 & m2r| + 1e-3

On-chip (per sample): one blob DMA carries fp8 a/p/n (chunk-major
[128, 4, 512]) plus 0xFF/0x00 mask bytes. Masking is a DVE bitwise-AND on
uint16 views (2x mode -> 4 fp8/cycle). Unmasked squares run split across
ScalarE/DVE/GpSimd. All matmuls are DoubleRow fp8 (K=256/step): an
indicator matmul reduces squares over c into P-maps [32, 3, 512], and the
U.V Gram accumulates band blocks into PSUM [128, 2, 136]. Both PSUM tiles
DMA straight to DRAM. Host extracts the 9 band diagonals (t3), applies
masks to the P-maps (t1/t2 - same O(B*H*W*9) class as the den counts),
and finishes min/relu/mean in f64.
"""

import os
import sys
from contextlib import ExitStack

import numpy as np

for _p in ("/opt/trn_rl_repo", "/root/.axon_site/_ro/trn_rl_repo"):
    if os.path.isdir(_p) and _p not in sys.path:
        sys.path.insert(0, _p)
        break

import ml_dtypes

import concourse.bass as bass
import concourse.mybir as mybir
import concourse.tile as tile

# This environment's walrus_driver allows only ONE sync-wait per instruction,
# while Tile freely aggregates several. Post-pass: move excess waits onto
# freshly inserted same-engine NOPs directly before the instruction.
_MAXW = 1


def _split_waits_pass(nc):
    n = 0
    for fn in nc.m.functions:
        for blk in fn.blocks:
            out = []
            changed = False
            for inst in blk.instructions:
                si = inst.sync_info
                waits = list(si.on_wait) if si is not None else []
                if len(waits) > _MAXW:
                    for i in range(0, len(waits) - _MAXW, _MAXW):
                        nop = mybir.InstNoOp(name=f"{inst.name}-wsplit{i}")
                        nop.engine = inst.engine
                        nop.sync_info = mybir.SyncInfo(
                            on_update=[], on_wait=waits[i : i + _MAXW]
                        )
                        out.append(nop)
                        n += 1
                    si.on_wait = waits[len(waits) - _MAXW :]
                    changed = True
                out.append(inst)
            if changed:
                blk.instructions = out
    return n


# concourse pins --enable-ldw-opt=false; enabling lets walrus elide/overlap
# redundant weight loads (the fp8 indicator weights repeat 6x per sample).
def _patch_ldw_opt():
    from concourse import bass_utils as _bu

    if getattr(_bu, "_ldw_opt_patched", False):
        return
    _orig = _bu.run_command

    def _run_command_ldwopt(cmd, *a, **kw):
        if isinstance(cmd, list):
            cmd = [
                "--enable-ldw-opt=true" if c == "--enable-ldw-opt=false" else c
                for c in cmd
            ]
        return _orig(cmd, *a, **kw)

    _bu.run_command = _run_command_ldwopt
    _bu._ldw_opt_patched = True


if os.environ.get("BASS_LDW_OPT", "0") == "1":
    _patch_ldw_opt()

F8 = mybir.dt.float8e4
U8 = mybir.dt.uint8
U16 = mybir.dt.uint16
F32 = mybir.dt.float32
BF16 = mybir.dt.bfloat16

B, C, H, W = 64, 16, 32, 512
NCORES = 8
S = B // NCORES          # samples per core
R = C * H                # 512 rows in (c,h), c-major: r = c*32 + h
NB = R // 128            # 4 partition chunks
JB = W // 128            # 4 w-blocks
NW = 136                 # gram window = 128 + 2*4
MARGIN = 0.15
SHIFT = 4
XB = 3 * 2048 + 3 * 512  # blob bytes/partition: a|p|n fp8 + 3 mask roles

_nc_cache = None


def build_nc(for_hw=True):
    DR = mybir.MatmulPerfMode.DoubleRow
    nc = bass.Bass()
    xin = nc.declare_dram_parameter("xin", [S, 128, XB], U8, isOutput=False)
    # indicator lhsT for the c-reduction: [k, i, m] = (m == k % 32), both i
    ind8 = nc.declare_dram_parameter("ind8", [128, 2, H], F8, isOutput=False)
    junk = nc.declare_dram_parameter("junk", [128, 2, W], F8, isOutput=False)
    # t3 gram bands; host reads the 9 diagonals col = m + 4 - off
    raw = nc.declare_dram_parameter("raw", [S, 128, 2, NW], BF16, isOutput=True)
    # P-maps: [h, (P1|P2p|P2n), w] = sum_c f^2, unmasked
    pmaps = nc.declare_dram_parameter("pmaps", [S, H, 3, W], BF16, isOutput=True)

    mult = mybir.AluOpType.mult
    band = mybir.AluOpType.bitwise_and
    Sq = mybir.ActivationFunctionType.Square
    Cp = mybir.ActivationFunctionType.Copy

    with tile.TileContext(nc) as tc, ExitStack() as ctx:
        const = ctx.enter_context(tc.tile_pool(name="const", bufs=1))
        io = ctx.enter_context(tc.tile_pool(name="io", bufs=3))
        um = ctx.enter_context(tc.tile_pool(name="um", bufs=3))
        sq = ctx.enter_context(tc.tile_pool(name="sq", bufs=3))
        outsb = ctx.enter_context(tc.tile_pool(name="outsb", bufs=3))
        indps = ctx.enter_context(tc.tile_pool(name="indps", bufs=2, space="PSUM"))
        gram = ctx.enter_context(tc.tile_pool(name="gram", bufs=2, space="PSUM"))

        ind_sb = const.tile([128, 2, H], F8)
        nc.sync.dma_start(out=ind_sb, in_=ind8[:])
        junk_sb = const.tile([128, 2, W], F8)
        nc.sync.dma_start(out=junk_sb, in_=junk[:])

        # PE prewarm: dense fp8 matmuls (no cross deps beyond 2-buf WAW) so
        # the clock ramps during the pipeline-fill phase.
        for _ in range(12):
            wt = indps.tile([H, 3, W], F32, tag="pm")
            nc.tensor.matmul(
                wt[:, 0, :], ind_sb, junk_sb, start=True, stop=True, perf_mode=DR
            )

        for s in range(S):
            # ---- one blob load: a|p|n fp8 + mask bytes ----
            xt = io.tile([128, XB], U8, tag="xt")
            nc.sync.dma_start(out=xt, in_=xin[s])

            a_f8 = xt[:, 0:2048].bitcast(F8).rearrange("p (j w) -> p j w", j=NB)
            p_f8 = xt[:, 2048:4096].bitcast(F8).rearrange("p (j w) -> p j w", j=NB)
            n_f8 = xt[:, 4096:6144].bitcast(F8).rearrange("p (j w) -> p j w", j=NB)
            a_u16 = xt[:, 0:2048].bitcast(U16).rearrange("p (j w) -> p j w", j=NB)
            pn_u16 = xt[:, 2048:6144].bitcast(U16).rearrange(
                "p (t j w) -> p j t w", t=2, j=NB
            )
            msk = xt[:, 6144:XB].bitcast(U16).rearrange("p (r w) -> p r w", r=3)

            # ---- masking: bitwise AND on uint16 views (DVE 2x) ----
            ubuf = um.tile([128, NB, W], F8, tag="ubuf")
            vw = um.tile([128, NB, 2, W + 8], F8, tag="vw")
            nc.vector.tensor_tensor(
                out=ubuf.bitcast(U16),
                in0=a_u16,
                in1=msk[:, 0, :].unsqueeze(1).broadcast_to((128, NB, W // 2)),
                op=band,
            )
            nc.vector.tensor_tensor(
                out=vw[:, :, :, 4 : W + 4].bitcast(U16),
                in0=pn_u16,
                in1=msk[:, 1:3, :].unsqueeze(1).broadcast_to((128, NB, 2, W // 2)),
                op=band,
            )
            # circular wrap columns of the gram window
            nc.vector.tensor_copy(
                out=vw[:, :, :, 0:4].bitcast(U16),
                in_=vw[:, :, :, W : W + 4].bitcast(U16),
            )
            nc.vector.tensor_copy(
                out=vw[:, :, :, W + 4 : W + 8].bitcast(U16),
                in_=vw[:, :, :, 4:8].bitcast(U16),
            )

            # ---- unmasked squares, split ACT / DVE / GpSimd ----
            u2 = sq.tile([128, NB, W], F8, tag="u2")
            p2 = sq.tile([128, NB, W], F8, tag="p2")
            n2 = sq.tile([128, NB, W], F8, tag="n2")
            nc.scalar.activation(out=u2, in_=a_f8, func=Sq)
            nc.scalar.activation(out=p2[:, 0:2, :], in_=p_f8[:, 0:2, :], func=Sq)
            nc.vector.tensor_tensor(
                out=p2[:, 2:4, :], in0=p_f8[:, 2:4, :], in1=p_f8[:, 2:4, :], op=mult
            )
            nc.gpsimd.tensor_tensor(out=n2, in0=n_f8, in1=n_f8, op=mult)

            # ---- c-reduction: DoubleRow indicator matmuls -> P-maps ----
            pm_ps = indps.tile([H, 3, W], F32, tag="pm")
            for t, sqt in enumerate((u2, p2, n2)):
                for q in range(2):
                    nc.tensor.matmul(
                        pm_ps[:, t, :],
                        ind_sb,
                        sqt[:, 2 * q : 2 * q + 2, :],
                        start=(q == 0),
                        stop=(q == 1),
                        perf_mode=DR,
                    )

            # ---- t3 gram: DoubleRow; per-pair matmuls share lhs weights ----
            num_ps = gram.tile([128, 2, NW], F32, tag="num")
            for j1 in range(JB):
                mb = slice(j1 * 128, (j1 + 1) * 128)
                wn = slice(j1 * 128, j1 * 128 + NW)
                for q in range(2):
                    for g in range(2):
                        nc.tensor.matmul(
                            num_ps[:, g, :],
                            ubuf[:, 2 * q : 2 * q + 2, mb],
                            vw[:, 2 * q : 2 * q + 2, g, wn],
                            start=(j1 == 0 and q == 0 and g == 0),
                            stop=(j1 == JB - 1 and q == 1 and g == 1),
                            perf_mode=DR,
                        )

            # ---- PSUM -> SBUF bf16 (halves out-DMA bytes), then DRAM ----
            psb = outsb.tile([128, 2, NW], BF16, tag="rawsb")
            pmb = outsb.tile([H, 3, W], BF16, tag="pmsb")
            nc.scalar.activation(out=psb, in_=num_ps, func=Cp)
            nc.scalar.activation(out=pmb, in_=pm_ps, func=Cp)
            nc.sync.dma_start(out=raw[s], in_=psb)
            nc.sync.dma_start(out=pmaps[s], in_=pmb)
    if for_hw:
        _split_waits_pass(nc)
    return nc


def _host_prep(a, p, n, ma, mp, mn):
    f8 = ml_dtypes.float8_e4m3

    def pack(x):
        # [B, C, H, W] f32 -> fp8 bytes [B, 128, NB*W]: partition p holds
        # rows r = j*128 + p, j-major along the free dim
        xr = np.asarray(x).reshape(B, R, W).astype(f8)
        return np.ascontiguousarray(
            xr.reshape(B, NB, 128, W).transpose(0, 2, 1, 3)
        ).reshape(B, 128, NB * W).view(np.uint8)

    def mbytes(m):
        mm = (np.asarray(m).reshape(B, H, W) != 0).astype(np.uint8) * np.uint8(0xFF)
        return np.tile(mm, (1, NB, 1))  # [B, 128, W], row p -> mask[p % 32]

    Mk = np.stack([mbytes(ma), mbytes(mp), mbytes(mn)], axis=2)  # [B,128,3,W]
    blob = np.concatenate(
        [pack(a), pack(p), pack(n), Mk.reshape(B, 128, 3 * W)], axis=2
    )  # [B, 128, XB] u8
    ind8 = np.zeros((128, 2, H), f8)
    ind8[np.arange(128), :, np.arange(128) % H] = f8(1.0)
    junk = np.zeros((128, 2, W), f8)
    in_maps = []
    for c in range(NCORES):
        sl = slice(c * S, (c + 1) * S)
        in_maps.append({"xin": blob[sl], "ind8": ind8, "junk": junk})
    return in_maps


def _host_finish(raw_all, pm_all, ma, mp, mn):
    # raw_all [B, 128, 2, NW] f32: t3 band blocks (diag col = m + 4 - off)
    # pm_all  [B, H, 3, W] f32: P1, P2p, P2n
    nb = raw_all.shape[0]
    raw64 = raw_all.astype(np.float64)
    pm64 = pm_all.astype(np.float64)
    m1 = np.asarray(ma).reshape(nb, H, W).astype(bool)
    m2s = [np.asarray(mp).reshape(nb, H, W).astype(bool),
           np.asarray(mn).reshape(nb, H, W).astype(bool)]
    P1 = pm64[:, :, 0, :]
    P2s = [pm64[:, :, 1, :], pm64[:, :, 2, :]]
    idx = np.arange(128)
    dists = np.empty((2 * SHIFT + 1, nb, 2), np.float64)
    for i, off in enumerate(range(-SHIFT, SHIFT + 1)):
        t3 = raw64[:, idx, :, idx + 4 - off].sum(axis=0)  # [nb, 2]
        for g in range(2):
            m2r = np.roll(m2s[g], off, axis=-1)
            both = (m1 & m2r).astype(np.float64)
            cnt = both.sum(axis=(1, 2))
            t1 = np.einsum("bhw,bhw->b", P1, both)
            t2 = np.einsum("bhw,bhw->b", np.roll(P2s[g], off, axis=-1), both)
            num = t1 + t2 - 2.0 * t3[:, g]
            dists[i, :, g] = num / (C * cnt + 0.001)
    d = dists.min(axis=0)  # [nb, 2]
    loss = np.maximum(d[:, 0] - d[:, 1] + MARGIN, 0.0)
    return np.array(loss.mean(), dtype=np.float32)


def kernel(a, p, n, ma, mp, mn):
    global _nc_cache
    from concourse import bass_utils

    if _nc_cache is None:
        _nc_cache = build_nc()
    nc = _nc_cache
    in_maps = _host_prep(a, p, n, ma, mp, mn)
    res = bass_utils.run_bass_kernel_spmd(nc, in_maps, core_ids=list(range(NCORES)))
    raw_all = np.concatenate([res.results[i]["raw"] for i in range(NCORES)], axis=0)
    pm_all = np.concatenate([res.results[i]["pmaps"] for i in range(NCORES)], axis=0)
    return _host_finish(raw_all, pm_all, ma, mp, mn)


# revision 12
# speedup vs baseline: 1.0967x; 1.0357x over previous
"""Bass/Trainium2 kernel for ExtendedTripletLoss (data-parallel over batch).

FP8 redesign. Math per sample and shift off in [-4,4], pair g in {ap, an}:
  num(off) = t1 + t2 - 2*t3
    t3 = sum_{r,w} U[r,w] V[r,w-off],  U = m1*f1, V = m2*f2   (fp8)
    t1 = sum_{h,w} m1[h,w] m2[h,w-off] P1[h,w],   P1 = sum_c f1^2
    t2 = sum_{h,w} m1[h,w] m2[h,w-off] P2[h,w-off], P2 = sum_c f2^2
  den(off) = C * |m1 & m2r| + 1e-3

On-chip (per sample): one blob DMA carries fp8 a/p/n (chunk-major
[128, 4, 512]) plus 0xFF/0x00 mask bytes. Masking is a DVE bitwise-AND on
uint16 views (2x mode -> 4 fp8/cycle). Unmasked squares run split across
ScalarE/DVE/GpSimd. All matmuls are DoubleRow fp8 (K=256/step): an
indicator matmul reduces squares over c into P-maps [32, 3, 512], and the
U.V Gram accumulates band blocks into PSUM [128, 2, 136]. Both PSUM tiles
DMA straight to DRAM. Host extracts the 9 band diagonals (t3), applies
masks to the P-maps (t1/t2 - same O(B*H*W*9) class as the den counts),
and finishes min/relu/mean in f64.
"""

import os
import sys
from contextlib import ExitStack

import numpy as np

for _p in ("/opt/trn_rl_repo", "/root/.axon_site/_ro/trn_rl_repo"):
    if os.path.isdir(_p) and _p not in sys.path:
        sys.path.insert(0, _p)
        break

import ml_dtypes

import concourse.bass as bass
import concourse.mybir as mybir
import concourse.tile as tile

# This environment's walrus_driver allows only ONE sync-wait per instruction,
# while Tile freely aggregates several. Post-pass: move excess waits onto
# freshly inserted same-engine NOPs directly before the instruction.
_MAXW = 1


def _split_waits_pass(nc):
    n = 0
    for fn in nc.m.functions:
        for blk in fn.blocks:
            out = []
            changed = False
            for inst in blk.instructions:
                si = inst.sync_info
                waits = list(si.on_wait) if si is not None else []
                if len(waits) > _MAXW:
                    for i in range(0, len(waits) - _MAXW, _MAXW):
                        nop = mybir.InstNoOp(name=f"{inst.name}-wsplit{i}")
                        nop.engine = inst.engine
                        nop.sync_info = mybir.SyncInfo(
                            on_update=[], on_wait=waits[i : i + _MAXW]
                        )
                        out.append(nop)
                        n += 1
                    si.on_wait = waits[len(waits) - _MAXW :]
                    changed = True
                out.append(inst)
            if changed:
                blk.instructions = out
    return n


# concourse pins --enable-ldw-opt=false; enabling lets walrus elide/overlap
# redundant weight loads (the fp8 indicator weights repeat 6x per sample).
def _patch_ldw_opt():
    from concourse import bass_utils as _bu

    if getattr(_bu, "_ldw_opt_patched", False):
        return
    _orig = _bu.run_command

    def _run_command_ldwopt(cmd, *a, **kw):
        if isinstance(cmd, list):
            cmd = [
                "--enable-ldw-opt=true" if c == "--enable-ldw-opt=false" else c
                for c in cmd
            ]
        return _orig(cmd, *a, **kw)

    _bu.run_command = _run_command_ldwopt
    _bu._ldw_opt_patched = True


if os.environ.get("BASS_LDW_OPT", "0") == "1":
    _patch_ldw_opt()

F8 = mybir.dt.float8e4
U8 = mybir.dt.uint8
U16 = mybir.dt.uint16
U32 = mybir.dt.uint32
F32 = mybir.dt.float32
BF16 = mybir.dt.bfloat16

B, C, H, W = 64, 16, 32, 512
NCORES = 8
S = B // NCORES          # samples per core
R = C * H                # 512 rows in (c,h), c-major: r = c*32 + h
NB = R // 128            # 4 partition chunks
JB = W // 128            # 4 w-blocks
NW = 136                 # gram window = 128 + 2*4
MARGIN = 0.15
SHIFT = 4
XB = 3 * 2048 + 3 * 512  # blob bytes/partition: a|p|n fp8 + 3 mask roles

_nc_cache = None


def build_nc(for_hw=True):
    DR = mybir.MatmulPerfMode.DoubleRow
    nc = bass.Bass()
    xin = nc.declare_dram_parameter("xin", [S, 128, XB], U8, isOutput=False)
    # indicator lhsT for the c-reduction: [k, i, m] = (m == k % 32), both i
    ind8 = nc.declare_dram_parameter("ind8", [128, 2, H], F8, isOutput=False)
    junk = nc.declare_dram_parameter("junk", [128, 2, W], F8, isOutput=False)
    # t3 gram bands; host reads the 9 diagonals col = m + 4 - off
    raw = nc.declare_dram_parameter("raw", [S, 128, 2, NW], BF16, isOutput=True)
    # P-maps: [h, (P1|P2p|P2n), w] = sum_c f^2, unmasked
    pmaps = nc.declare_dram_parameter("pmaps", [S, H, 3, W], BF16, isOutput=True)

    mult = mybir.AluOpType.mult
    band = mybir.AluOpType.bitwise_and
    Sq = mybir.ActivationFunctionType.Square
    Cp = mybir.ActivationFunctionType.Copy

    with tile.TileContext(nc) as tc, ExitStack() as ctx:
        const = ctx.enter_context(tc.tile_pool(name="const", bufs=1))
        io = ctx.enter_context(tc.tile_pool(name="io", bufs=1))
        um = ctx.enter_context(tc.tile_pool(name="um", bufs=3))
        sq = ctx.enter_context(tc.tile_pool(name="sq", bufs=3))
        outsb = ctx.enter_context(tc.tile_pool(name="outsb", bufs=3))
        indps = ctx.enter_context(tc.tile_pool(name="indps", bufs=2, space="PSUM"))
        gram = ctx.enter_context(tc.tile_pool(name="gram", bufs=2, space="PSUM"))

        ind_sb = const.tile([128, 2, H], F8)
        nc.sync.dma_start(out=ind_sb, in_=ind8[:])
        junk_sb = const.tile([128, 2, W], F8)
        nc.sync.dma_start(out=junk_sb, in_=junk[:])

        # Pre-issue ALL sample loads so the sync queue never gates a load on
        # a previous sample's compute (loads stream; outputs trail behind).
        xts = []
        for s in range(S):
            xt = io.tile([128, XB], U8, tag=f"xt{s}")
            nc.sync.dma_start(out=xt, in_=xin[s])
            xts.append(xt)

        # PE prewarm: dense fp8 matmuls (no cross deps beyond 2-buf WAW) so
        # the clock ramps during the pipeline-fill phase.
        for _ in range(12):
            wt = indps.tile([H, 3, W], F32, tag="pm")
            nc.tensor.matmul(
                wt[:, 0, :], ind_sb, junk_sb, start=True, stop=True, perf_mode=DR
            )

        for s in range(S):
            xt = xts[s]
            a_f8 = xt[:, 0:2048].bitcast(F8).rearrange("p (j w) -> p j w", j=NB)
            p_f8 = xt[:, 2048:4096].bitcast(F8).rearrange("p (j w) -> p j w", j=NB)
            n_f8 = xt[:, 4096:6144].bitcast(F8).rearrange("p (j w) -> p j w", j=NB)
            a_u32 = xt[:, 0:2048].bitcast(U32).rearrange("p (j w) -> p j w", j=NB)
            pn_u32 = xt[:, 2048:6144].bitcast(U32).rearrange(
                "p (t j w) -> p j t w", t=2, j=NB
            )
            msk = xt[:, 6144:XB].bitcast(U32).rearrange("p (r w) -> p r w", r=3)

            # ---- masking: bitwise AND on uint32 views (4 fp8/cycle) ----
            ubuf = um.tile([128, NB, W], F8, tag="ubuf")
            vw = um.tile([128, NB, 2, W + 8], F8, tag="vw")
            nc.vector.tensor_tensor(
                out=ubuf.bitcast(U32),
                in0=a_u32,
                in1=msk[:, 0, :].unsqueeze(1).broadcast_to((128, NB, W // 4)),
                op=band,
            )
            nc.vector.tensor_tensor(
                out=vw[:, :, :, 4 : W + 4].bitcast(U32),
                in0=pn_u32,
                in1=msk[:, 1:3, :].unsqueeze(1).broadcast_to((128, NB, 2, W // 4)),
                op=band,
            )
            # circular wrap columns of the gram window
            nc.vector.tensor_copy(
                out=vw[:, :, :, 0:4].bitcast(U32),
                in_=vw[:, :, :, W : W + 4].bitcast(U32),
            )
            nc.vector.tensor_copy(
                out=vw[:, :, :, W + 4 : W + 8].bitcast(U32),
                in_=vw[:, :, :, 4:8].bitcast(U32),
            )

            # ---- unmasked squares: ACT takes a^2+p^2, GpSimd n^2 ----
            u2 = sq.tile([128, NB, W], F8, tag="u2")
            p2 = sq.tile([128, NB, W], F8, tag="p2")
            n2 = sq.tile([128, NB, W], F8, tag="n2")
            nc.gpsimd.tensor_tensor(out=n2, in0=n_f8, in1=n_f8, op=mult)
            nc.scalar.activation(out=u2, in_=a_f8, func=Sq)
            nc.scalar.activation(out=p2, in_=p_f8, func=Sq)

            # ---- c-reduction: DoubleRow indicator matmuls -> P-maps ----
            pm_ps = indps.tile([H, 3, W], F32, tag="pm")
            for t, sqt in enumerate((u2, p2, n2)):
                for q in range(2):
                    nc.tensor.matmul(
                        pm_ps[:, t, :],
                        ind_sb,
                        sqt[:, 2 * q : 2 * q + 2, :],
                        start=(q == 0),
                        stop=(q == 1),
                        perf_mode=DR,
                    )

            # ---- t3 gram: DoubleRow; per-pair matmuls share lhs weights ----
            num_ps = gram.tile([128, 2, NW], F32, tag="num")
            for j1 in range(JB):
                mb = slice(j1 * 128, (j1 + 1) * 128)
                wn = slice(j1 * 128, j1 * 128 + NW)
                for q in range(2):
                    for g in range(2):
                        nc.tensor.matmul(
                            num_ps[:, g, :],
                            ubuf[:, 2 * q : 2 * q + 2, mb],
                            vw[:, 2 * q : 2 * q + 2, g, wn],
                            start=(j1 == 0 and q == 0 and g == 0),
                            stop=(j1 == JB - 1 and q == 1 and g == 1),
                            perf_mode=DR,
                        )

            # ---- PSUM -> SBUF bf16 (halves out-DMA bytes), then DRAM ----
            psb = outsb.tile([128, 2, NW], BF16, tag="rawsb")
            pmb = outsb.tile([H, 3, W], BF16, tag="pmsb")
            nc.scalar.activation(out=psb, in_=num_ps, func=Cp)
            nc.vector.tensor_copy(out=pmb, in_=pm_ps)
            nc.sync.dma_start(out=raw[s], in_=psb)
            nc.sync.dma_start(out=pmaps[s], in_=pmb)
    if for_hw:
        _split_waits_pass(nc)
    return nc


def _host_prep(a, p, n, ma, mp, mn):
    f8 = ml_dtypes.float8_e4m3

    def pack(x):
        # [B, C, H, W] f32 -> fp8 bytes [B, 128, NB*W]: partition p holds
        # rows r = j*128 + p, j-major along the free dim
        xr = np.asarray(x).reshape(B, R, W).astype(f8)
        return np.ascontiguousarray(
            xr.reshape(B, NB, 128, W).transpose(0, 2, 1, 3)
        ).reshape(B, 128, NB * W).view(np.uint8)

    def mbytes(m):
        mm = (np.asarray(m).reshape(B, H, W) != 0).astype(np.uint8) * np.uint8(0xFF)
        return np.tile(mm, (1, NB, 1))  # [B, 128, W], row p -> mask[p % 32]

    Mk = np.stack([mbytes(ma), mbytes(mp), mbytes(mn)], axis=2)  # [B,128,3,W]
    blob = np.concatenate(
        [pack(a), pack(p), pack(n), Mk.reshape(B, 128, 3 * W)], axis=2
    )  # [B, 128, XB] u8
    ind8 = np.zeros((128, 2, H), f8)
    ind8[np.arange(128), :, np.arange(128) % H] = f8(1.0)
    junk = np.zeros((128, 2, W), f8)
    in_maps = []
    for c in range(NCORES):
        sl = slice(c * S, (c + 1) * S)
        in_maps.append({"xin": blob[sl], "ind8": ind8, "junk": junk})
    return in_maps


def _host_finish(raw_all, pm_all, ma, mp, mn):
    # raw_all [B, 128, 2, NW] f32: t3 band blocks (diag col = m + 4 - off)
    # pm_all  [B, H, 3, W] f32: P1, P2p, P2n
    nb = raw_all.shape[0]
    raw64 = raw_all.astype(np.float64)
    pm64 = pm_all.astype(np.float64)
    m1 = np.asarray(ma).reshape(nb, H, W).astype(bool)
    m2s = [np.asarray(mp).reshape(nb, H, W).astype(bool),
           np.asarray(mn).reshape(nb, H, W).astype(bool)]
    P1 = pm64[:, :, 0, :]
    P2s = [pm64[:, :, 1, :], pm64[:, :, 2, :]]
    idx = np.arange(128)
    dists = np.empty((2 * SHIFT + 1, nb, 2), np.float64)
    for i, off in enumerate(range(-SHIFT, SHIFT + 1)):
        t3 = raw64[:, idx, :, idx + 4 - off].sum(axis=0)  # [nb, 2]
        for g in range(2):
            m2r = np.roll(m2s[g], off, axis=-1)
            both = (m1 & m2r).astype(np.float64)
            cnt = both.sum(axis=(1, 2))
            t1 = np.einsum("bhw,bhw->b", P1, both)
            t2 = np.einsum("bhw,bhw->b", np.roll(P2s[g], off, axis=-1), both)
            num = t1 + t2 - 2.0 * t3[:, g]
            dists[i, :, g] = num / (C * cnt + 0.001)
    d = dists.min(axis=0)  # [nb, 2]
    loss = np.maximum(d[:, 0] - d[:, 1] + MARGIN, 0.0)
    return np.array(loss.mean(), dtype=np.float32)


def kernel(a, p, n, ma, mp, mn):
    global _nc_cache
    from concourse import bass_utils

    if _nc_cache is None:
        _nc_cache = build_nc()
    nc = _nc_cache
    in_maps = _host_prep(a, p, n, ma, mp, mn)
    res = bass_utils.run_bass_kernel_spmd(nc, in_maps, core_ids=list(range(NCORES)))
    raw_all = np.concatenate([res.results[i]["raw"] for i in range(NCORES)], axis=0)
    pm_all = np.concatenate([res.results[i]["pmaps"] for i in range(NCORES)], axis=0)
    return _host_finish(raw_all, pm_all, ma, mp, mn)
